# revision 42
# baseline (speedup 1.0000x reference)
"""Trainium2 Bass kernel for Falcon-7B MQA flash-decode attention block.

Geometry (hardcoded from the problem spec):
  hidden [1, 32, 4544], w_qkv [4672, 4544] (71 q heads + 1 k + 1 v, hd=64),
  kv cache [4, 1, 32, 2048, 64], masks [4, 1, 32, 2048], w_dense [4544, 4544].

Sharding across 8 NeuronCores:
  - users (32) are data-parallel, 4 per core: each core holds its users' KV.
  - w_qkv / w_dense are tensor-parallel column-split 8 ways; an AllToAll
    redistributes the fused QKV activations from column-shards to user-shards,
    one AllGather collects attention outputs for the dense matmul.
  - all matmul operands are bf16 (host-cast); PSUM accumulation stays fp32.
    bf16 is 4x faster on the PE per moving row and halves HBM traffic.
  - softmax uses the shift-invariant (max-free) formulation; the attention
    mask rides along as contraction row 64 of kT with q row 64 = 8.0, so the
    ACT exp's 1/8 scale returns exactly qk/8 + mask. No separate mask ops.
  - scores are packed 7 per PSUM bank at stride 71; exp batches 14 s-tiles
    (2 banks) per ACT op to amortize the ~185ns fixed ACT latency.
  - attnT for the dense matmul comes from one xbar DMA transpose
    (dma_start_transpose) of the gathered attention, not PE transposes.
  - dummy matmuls keep the PE p-state at 2.4 GHz across the collective idle
    windows (cold-start matmuls cost 3.7x); small guard copies sequence the
    serial DMA device so critical transfers are not queued behind bulk
    prefetch (the DMA device grants FIFO by readiness time).

Host-side prep is layout-only (transposes / packing / dtype casts).
"""

import sys

if "/opt/trn_rl_repo" not in sys.path:
    sys.path.insert(0, "/opt/trn_rl_repo")

import numpy as np

import concourse.bacc as bacc
import concourse.bass as bass
import concourse.mybir as mybir
import concourse.tile as tile
from concourse.bass_utils import run_bass_kernel_spmd
from concourse.masks import make_identity

F32 = mybir.dt.float32
BF16 = mybir.dt.bfloat16

NCORES = 8
U = 32          # users total
UPC = 4         # users per core
HID = 4544
NH = 71         # query heads
HD = 64
HPC = 10        # heads per core in the padded qkv column split (8*10*64 = 5120)
NCOL = HPC * HD         # 640 fused columns per core
DN = HID // NCORES      # 568 dense output columns per core
S = 8192                # total cached tokens per user (4 chunks x 2048)
NT = S // 128           # 64 s-tiles of 128
KT = 36                 # k-tiles over HID (zero-padded to 4608 rows)
WQS = 6                 # wq k-tiles per DMA slab (6 slabs of 6)
QC = NCOL // 4          # 160 fused columns per PSUM quadrant
DC = DN // 4            # 142 dense columns per PSUM quadrant
EG = (14, 14, 14, 14, 8)  # exp batch sizes over the 64 s-tiles

LAST_RESULT = None
_prog = None


def _build():
    nc = bacc.Bacc("TRN2", target_bir_lowering=False, debug=False,
                   num_devices=NCORES)

    # host-packed inputs (see kernel() below)
    hT = nc.dram_tensor("hT", [128, KT, U], BF16, kind="ExternalInput")
    wq = nc.dram_tensor("wq", [128, KT, NCOL], BF16, kind="ExternalInput")
    wd = nc.dram_tensor("wd", [128, KT, DN], BF16, kind="ExternalInput")
    kTc = nc.dram_tensor("kTc", [UPC, HD + 1, S], BF16, kind="ExternalInput")
    vc = nc.dram_tensor("vc", [UPC, 128, NT, HD + 1], BF16,
                        kind="ExternalInput")
    muT = nc.dram_tensor("muT", [HD, UPC, HD], F32, kind="ExternalInput")
    outc = nc.dram_tensor("outc", [U, DN], F32, kind="ExternalOutput")

    with tile.TileContext(nc) as tc:
        with (
            tc.tile_pool(name="const", bufs=1) as const,
            tc.tile_pool(name="wpool", bufs=3) as wpool,
            tc.tile_pool(name="kpool", bufs=3) as kpool,
            tc.tile_pool(name="vpool", bufs=3) as vpool,
            tc.tile_pool(name="ppool", bufs=2) as ppool,
            tc.tile_pool(name="upool", bufs=2) as upool,
            tc.tile_pool(name="pqpool", bufs=1, space="PSUM") as pqpool,
            tc.tile_pool(name="psc", bufs=3, space="PSUM") as pscpool,
            tc.tile_pool(name="pvpool", bufs=1, space="PSUM") as pvpool,
            tc.tile_pool(name="dram", bufs=1, space="DRAM") as dram,
        ):
            identity = const.tile([128, 128], F32)
            make_identity(nc, identity)

            # warm the PE p-state during the initial weight-DMA wait: ~3us of
            # continuous dummy matmuls take the clock 0.65 -> 2.4 GHz before
            # the first real QKV matmul lands
            wtile = const.tile([128, 128], BF16)
            nc.vector.memset(wtile, 0.0)
            ps_w = pscpool.tile([128, 2, 512], F32, tag="sg", name="ps_w")
            for w in range(30):
                nc.tensor.matmul(ps_w[0:1, 0, 0:128], wtile[:, 0:1],
                                 wtile[:, 0:128], start=True, stop=True)

            # ---------------- phase A: fused QKV projection ----------------
            hT_all = const.tile([128, KT, U], BF16)
            nc.sync.dma_start(out=hT_all, in_=hT[:, :, :])
            muT_sb = const.tile([HD, UPC, HD], F32)
            # gate behind the hT load: muT is only needed after the AllToAll
            nc.vector.tensor_copy(out=muT_sb[0:1, 0:1, 0:1],
                                  in_=hT_all[0:1, 0:1, 0:1])
            nc.scalar.dma_start(out=muT_sb, in_=muT[:, :, :])

            psQ = pqpool.tile([128, QC], F32, tag="bank", name="psQ")
            # slab sizes: first split 2+4 for an early start, last two small
            # (5+1) so the PE tail after the final transfer is 4 matmuls
            # instead of 24
            SLABS = [(0, 6), (6, 6), (12, 6), (18, 6), (24, 6), (30, 5),
                     (35, 1)]
            wslabs = []
            for si, (t0s, tns) in enumerate(SLABS):
                wslab = wpool.tile([128, tns, NCOL], BF16, tag="w",
                                   name="wslab", uniquify=True)
                if si == 0:
                    nc.sync.dma_start(out=wslab[:, 0:2, :],
                                      in_=wq[:, 0:2, :])
                    nc.sync.dma_start(out=wslab[:, 2:tns, :],
                                      in_=wq[:, 2:tns, :])
                else:
                    nc.sync.dma_start(out=wslab,
                                      in_=wq[:, t0s:t0s + tns, :])
                wslabs.append(wslab)
            for si, (t0s, tns) in enumerate(SLABS):
                for t6 in range(tns):
                    t = t0s + t6
                    lhs = hT_all[:, t, :]
                    for j in range(4):
                        nc.tensor.matmul(
                            psQ[32 * j:32 * j + 32, :], lhs,
                            wslabs[si][:, t6, QC * j:QC * (j + 1)],
                            start=(t == 0), stop=(t == KT - 1),
                            tile_position=(0, 32 * j))

            fq_sb = const.tile([128, QC], BF16)
            nc.scalar.activation(out=fq_sb, in_=psQ[:, :],
                                 func=mybir.ActivationFunctionType.Copy)

            fused_x = dram.tile([U, NCOL], BF16)
            fused_x_ji = bass.AP(
                tensor=fused_x.tensor, offset=fused_x.offset,
                ap=[[QC, 4], [NCOL, U], [1, QC]])
            # ACT-ring store: the SP ring is busy prefetching KV
            nc.scalar.dma_start(out=fused_x_ji, in_=fq_sb)
            # block d of the flat input (users 4d..4d+3) goes to core d
            fused_loc = dram.tile([NCORES, UPC, NCOL], BF16)
            nc.gpsimd.collective_compute(
                "AllToAll", mybir.AluOpType.bypass,
                replica_groups=[list(range(NCORES))],
                ins=[fused_x.opt()], outs=[fused_loc.opt()])

            # one strided gather for all 4 local users: (head, user, d)
            q_bf = const.tile([80, UPC, HD], BF16)
            for i in range(UPC):
                nc.scalar.dma_start(
                    out=q_bf[:, i, :],
                    in_=bass.AP(
                        tensor=fused_loc.tensor,
                        offset=fused_loc.offset + i * NCOL,
                        ap=[[UPC * NCOL, NCORES], [HD, HPC], [1, HD]]))
            q_f32 = const.tile([NH + 1, UPC, HD], F32)
            for i in range(UPC):
                nc.vector.tensor_copy(out=q_f32[:, i, :],
                                      in_=q_bf[0:NH + 1, i, :])
            wd_sb = const.tile([128, KT, DN], BF16)
            # dense-weight slabs 0/1: gated on the last q gather, racing the
            # user-3 KV loads for the post-A2A DMA slots
            for g in range(2):
                nc.vector.tensor_copy(
                    out=wd_sb[0:1, 9 * g:9 * g + 1, 0:1],
                    in_=q_bf[0:1, UPC - 1, 0:1])
                nc.sync.dma_start(
                    out=wd_sb[:, 9 * g:9 * (g + 1), :],
                    in_=wd[:, 9 * g:9 * (g + 1), :])
            vcur_all = const.tile([1, UPC, HD + 1], BF16)  # [v_cur | 1]
            nc.scalar.dma_start(
                out=vcur_all[:, :, 0:HD],
                in_=fused_loc[7, :, 2 * HD:3 * HD][None, :, :])
            nc.vector.memset(vcur_all[:, :, HD:HD + 1], 1.0)

            # ---------------- phase C: per-user flash-decode attention ------
            HIDP = KT * 128  # attn padded to 4608 so xbar tiles divide
            attn_c = dram.tile([UPC, HIDP], BF16, name="attn_c")
            zero4 = const.tile([UPC, HD], BF16)
            nc.vector.memset(zero4, 0.0)
            nc.scalar.dma_start(
                out=bass.AP(tensor=attn_c.tensor,
                            offset=attn_c.offset + HID,
                            ap=[[HIDP, UPC], [1, HD]]),
                in_=zero4)
            attn_ag = dram.tile([NCORES, UPC, HIDP], BF16,
                                addr_space="Shared", name="attn_ag")

            for i in range(UPC):
                # [k^T | mask row]: contraction row 64 carries the mask; the
                # q side puts 8.0 there so exp's 1/8 scale yields qk/8 + m
                kT_sb = kpool.tile([HD + 1, S], BF16, tag="kT", name="kT_sb")
                vones = vpool.tile([128, NT, HD + 1], BF16, tag="v",
                                   name="vones")
                if i < 3:
                    # guard: users 0/1/2 KV loads start once phase A's weight
                    # traffic is done (fq_sb written): they fill the AllToAll
                    # window on the otherwise idle DMA device
                    nc.vector.tensor_copy(out=kT_sb[0:1, 0:1],
                                          in_=fq_sb[0:1, 0:1])
                    nc.vector.tensor_copy(out=vones[0:1, 0:1, 0:1],
                                          in_=fq_sb[0:1, 0:1])
                else:
                    # user-3 KV right after the post-A2A q gathers
                    nc.vector.tensor_copy(out=kT_sb[0:1, 0:1],
                                          in_=q_bf[0:1, UPC - 1, 0:1])
                    nc.vector.tensor_copy(out=vones[0:1, 0:1, 0:1],
                                          in_=q_bf[0:1, UPC - 1, 0:1])

                nc.sync.dma_start(out=kT_sb, in_=kTc[i])
                nc.sync.dma_start(out=vones, in_=vc[i])
                # q heads 0..70 plus the shared k head at row 71, transposed
                ps_qT = pqpool.tile([HD, NH + 1], F32, tag="bank",
                                     name="ps_qT", uniquify=True)
                nc.tensor.transpose(ps_qT, q_f32[0:NH + 1, i, :],
                                    identity[0:NH + 1, 0:NH + 1])
                qkT = upool.tile([HD, NH + 1], F32, tag="qkT", name="qkT")
                nc.vector.tensor_copy(out=qkT, in_=ps_qT)

                # rotary as a matmul; row 64 = 8.0 scales the kT mask row
                ps_rot = pqpool.tile([HD, NH + 1], F32, tag="bank",
                                      name="ps_rot", uniquify=True)
                nc.tensor.matmul(ps_rot, muT_sb[:, i, :], qkT,
                                 start=True, stop=True)
                qTr = upool.tile([HD + 1, NH + 1], BF16, tag="qTr",
                                 name="qTr")
                nc.vector.tensor_copy(out=qTr[0:HD, :], in_=ps_rot)
                nc.vector.memset(qTr[HD:HD + 1, :], 8.0)

                # current-token score for all heads: [1, 71] (no mask row)
                ps_sc = pqpool.tile([1, NH], F32, tag="bank", name="ps_sc",
                                    uniquify=True)
                nc.tensor.matmul(ps_sc, qTr[0:HD, NH:NH + 1],
                                 qTr[0:HD, 0:NH], start=True, stop=True)
                curw = upool.tile([1, NH], BF16, tag="curw", name="curw")
                nc.scalar.activation(out=curw, in_=ps_sc,
                                     func=mybir.ActivationFunctionType.Exp,
                                     scale=0.125)

                # scores + exp over the 64 s-tiles, 7 tiles per PSUM bank at
                # stride 71, one batched exp per 2-bank group
                pT_all = ppool.tile([128, NT, NH], BF16, tag="pT",
                                    name="pT_all")
                pv = pvpool.tile([NH, HD + 1], F32, tag="pv", name="pv")
                t0 = 0
                for gi, gn in enumerate(EG):
                    ps_g = pscpool.tile([128, 2, 512], F32, tag="sg",
                                        name="ps_g")
                    for k in range(gn):
                        t = t0 + k
                        nc.tensor.matmul(
                            ps_g[:, k // 7, (k % 7) * NH:(k % 7 + 1) * NH],
                            kT_sb[:, t * 128:(t + 1) * 128],
                            qTr[:, 0:NH], start=True, stop=True)
                    if gn == 14:
                        nc.scalar.activation(
                            out=pT_all[:, t0:t0 + 14, :],
                            in_=ps_g[:, :, 0:7 * NH].rearrange(
                                "p b (k h) -> p b k h", h=NH),
                            func=mybir.ActivationFunctionType.Exp,
                            scale=0.125)
                    else:
                        for b in range((gn + 6) // 7):
                            bn = min(7, gn - 7 * b)
                            nc.scalar.activation(
                                out=pT_all[:, t0 + 7 * b:t0 + 7 * b + bn, :],
                                in_=ps_g[:, b, 0:bn * NH].rearrange(
                                    "p (k h) -> p k h", h=NH),
                                func=mybir.ActivationFunctionType.Exp,
                                scale=0.125)
                    t0 += gn

                # PV with fused row-sum via the ones column
                for t in range(NT):
                    nc.tensor.matmul(pv, pT_all[:, t, :], vones[:, t, :],
                                     start=(t == 0), stop=False)
                nc.tensor.matmul(pv, curw, vcur_all[:, i, :], start=False,
                                 stop=True)

                linv = upool.tile([NH, 1], F32, tag="linv", name="linv")
                nc.vector.reciprocal(out=linv, in_=pv[:, HD:HD + 1])
                attn_sb = upool.tile([NH, HD], BF16, tag="attn",
                                     name="attn_sb")
                nc.vector.tensor_scalar_mul(attn_sb, pv[:, 0:HD], linv)
                nc.scalar.dma_start(
                    out=bass.AP(tensor=attn_c.tensor,
                                offset=attn_c.offset + i * HIDP,
                                ap=[[HD, NH], [1, HD]]),
                    in_=attn_sb)

            # last dense-weight slab: bounce off the user-3 attn store so
            # its 3.6us transfer cannot delay the store (and the AllGather)
            bounce = const.tile([1, HD], BF16)
            nc.sync.dma_start(
                out=bounce,
                in_=bass.AP(tensor=attn_c.tensor,
                            offset=attn_c.offset + (UPC - 1) * HIDP,
                            ap=[[HD, 1], [1, HD]]))
            for g in (2, 3):
                nc.vector.tensor_copy(out=wd_sb[0:1, 9 * g:9 * g + 1, 0:1],
                                      in_=bounce[0:1, 0:1])
                nc.sync.dma_start(out=wd_sb[:, 9 * g:9 * (g + 1), :],
                                  in_=wd[:, 9 * g:9 * (g + 1), :])

            nc.gpsimd.collective_compute(
                "AllGather", mybir.AluOpType.bypass,
                replica_groups=[list(range(NCORES))],
                ins=[attn_c.opt()], outs=[attn_ag.opt()])

            # keep the PE p-state warm across the AllGather idle window so
            # the dense matmuls start at 2.4 GHz instead of 0.65 GHz
            ps_w2 = pscpool.tile([128, 2, 512], F32, tag="sg",
                                 name="ps_w2")
            for w in range(595):
                nc.tensor.matmul(ps_w2[0:1, 0, 0:128], wtile[:, 0:1],
                                 wtile[:, 0:128], start=True, stop=True)

            # ---------------- phase D: dense output projection --------------
            # attnT via two xbar DMA transposes of the gathered activations
            # (halves, so the dense matmuls start on k-tiles 0-17 while the
            # second half is still in flight)
            attnT = const.tile([128, KT, U], BF16)
            attn_flat = attn_ag.rearrange("c j n -> (c j) n")
            HK = KT // 2
            nc.sync.dma_start_transpose(
                out=attnT[:, 0:HK, :], in_=attn_flat[:, 0:HK * 128])
            nc.sync.dma_start_transpose(
                out=attnT[:, HK:KT, :], in_=attn_flat[:, HK * 128:KT * 128])

            psD = pqpool.tile([128, DC], F32, tag="bank", name="psD")
            for t in range(KT):
                for j in range(4):
                    nc.tensor.matmul(psD[32 * j:32 * j + 32, :],
                                     attnT[:, t, :],
                                     wd_sb[:, t, DC * j:DC * (j + 1)],
                                     start=(t == 0), stop=(t == KT - 1),
                                     tile_position=(0, 32 * j))

            outD = const.tile([128, DC], F32)
            nc.vector.tensor_copy(out=outD, in_=psD[:, :])
            outc_ji = bass.AP(
                tensor=outc.ap().tensor, offset=0,
                ap=[[DC, 4], [DN, U], [1, DC]])
            nc.scalar.dma_start(out=outc_ji, in_=outD)

    nc.compile()
    return nc


def _rot_mat(cos_u, sin_u):
    """M such that M @ x = x*cos + rotate_half(x)*sin, for one user."""
    m = np.zeros((HD, HD), np.float32)
    np.fill_diagonal(m, cos_u)
    half = HD // 2
    for r in range(half):
        m[r, r + half] += -sin_u[r]
        m[r + half, r] += sin_u[r + half]
    return m


def kernel(hidden_states, cos, sin, k_cache, v_cache, attn_masks, w_qkv,
           w_dense, trace=False):
    global _prog, LAST_RESULT
    import ml_dtypes

    bf16 = ml_dtypes.bfloat16
    if _prog is None:
        _prog = _build()

    hidden_states = np.asarray(hidden_states, np.float32)
    cos = np.asarray(cos, np.float32)
    sin = np.asarray(sin, np.float32)
    k_cache = np.asarray(k_cache, np.float32)
    v_cache = np.asarray(v_cache, np.float32)
    attn_masks = np.asarray(attn_masks, np.float32)
    w_qkv = np.asarray(w_qkv, np.float32)
    w_dense = np.asarray(w_dense, np.float32)

    def pack_k(m, ncol):
        """[4544, ncol] -> [128, 36, ncol] bf16, zero-padded to 4608 rows."""
        p = np.zeros((KT * 128, ncol), np.float32)
        p[:m.shape[0]] = m
        return np.ascontiguousarray(
            p.reshape(KT, 128, ncol).transpose(1, 0, 2).astype(bf16))

    hT = pack_k(hidden_states[0].T, U)                       # [128, 36, 32]
    wqT = np.zeros((HID, NCORES * NCOL), np.float32)
    wqT[:, :w_qkv.shape[0]] = w_qkv.T
    wdT = w_dense.T                                          # [4544, 4544]

    in_maps = []
    for c in range(NCORES):
        us = slice(UPC * c, UPC * (c + 1))
        k_u = np.moveaxis(k_cache[:, 0, us], 1, 0).reshape(UPC, S, HD)
        m_u = np.moveaxis(attn_masks[:, 0, us], 1, 0).reshape(UPC, S)
        kT_u = np.concatenate(
            [np.transpose(k_u, (0, 2, 1)), m_u[:, None, :]], axis=1)
        v_u = np.moveaxis(v_cache[:, 0, us], 1, 0).reshape(UPC, NT, 128, HD)
        vones = np.concatenate(
            [v_u, np.ones((UPC, NT, 128, 1), np.float32)], axis=3)
        muT = np.stack([
            _rot_mat(cos[0, u, 0], sin[0, u, 0]).T
            for u in range(UPC * c, UPC * (c + 1))
        ])                                                   # [4, 64, 64]
        in_maps.append({
            "hT": hT,
            "wq": pack_k(wqT[:, NCOL * c:NCOL * (c + 1)], NCOL),
            "wd": pack_k(wdT[:, DN * c:DN * (c + 1)], DN),
            "kTc": np.ascontiguousarray(kT_u.astype(bf16)),
            "vc": np.ascontiguousarray(
                vones.transpose(0, 2, 1, 3).astype(bf16)),
            "muT": np.ascontiguousarray(
                np.transpose(muT, (1, 0, 2)).astype(np.float32)),
        })

    res = run_bass_kernel_spmd(_prog, in_maps, list(range(NCORES)),
                               trace=trace)
    LAST_RESULT = res
    out = np.concatenate([res.results[c]["outc"] for c in range(NCORES)],
                         axis=1)                             # [32, 4544]
    return out[None].astype(np.float32)


# revision 43
# speedup vs baseline: 1.0131x; 1.0131x over previous
"""Trainium2 Bass kernel for Falcon-7B MQA flash-decode attention block.

Geometry (hardcoded from the problem spec):
  hidden [1, 32, 4544], w_qkv [4672, 4544] (71 q heads + 1 k + 1 v, hd=64),
  kv cache [4, 1, 32, 2048, 64], masks [4, 1, 32, 2048], w_dense [4544, 4544].

Sharding across 8 NeuronCores:
  - users (32) are data-parallel, 4 per core: each core holds its users' KV.
  - w_qkv / w_dense are tensor-parallel column-split 8 ways; an AllToAll
    redistributes the fused QKV activations from column-shards to user-shards,
    one AllGather collects attention outputs for the dense matmul.
  - all matmul operands are bf16 (host-cast); PSUM accumulation stays fp32.
    bf16 is 4x faster on the PE per moving row and halves HBM traffic.
  - softmax uses the shift-invariant (max-free) formulation; the attention
    mask rides along as contraction row 64 of kT with q row 64 = 8.0, so the
    ACT exp's 1/8 scale returns exactly qk/8 + mask. No separate mask ops.
  - scores are packed 7 per PSUM bank at stride 71; exp batches 14 s-tiles
    (2 banks) per ACT op to amortize the ~185ns fixed ACT latency.
  - attnT for the dense matmul comes from one xbar DMA transpose
    (dma_start_transpose) of the gathered attention, not PE transposes.
  - dummy matmuls keep the PE p-state at 2.4 GHz across the collective idle
    windows (cold-start matmuls cost 3.7x); small guard copies sequence the
    serial DMA device so critical transfers are not queued behind bulk
    prefetch (the DMA device grants FIFO by readiness time).

Host-side prep is layout-only (transposes / packing / dtype casts).
"""

import sys

if "/opt/trn_rl_repo" not in sys.path:
    sys.path.insert(0, "/opt/trn_rl_repo")

import numpy as np

import concourse.bacc as bacc
import concourse.bass as bass
import concourse.mybir as mybir
import concourse.tile as tile
from concourse.bass_utils import run_bass_kernel_spmd
from concourse.masks import make_identity

F32 = mybir.dt.float32
BF16 = mybir.dt.bfloat16

NCORES = 8
U = 32          # users total
UPC = 4         # users per core
HID = 4544
NH = 71         # query heads
HD = 64
HPC = 10        # heads per core in the padded qkv column split (8*10*64 = 5120)
NCOL = HPC * HD         # 640 fused columns per core
DN = HID // NCORES      # 568 dense output columns per core
S = 8192                # total cached tokens per user (4 chunks x 2048)
NT = S // 128           # 64 s-tiles of 128
KT = 36                 # k-tiles over HID (zero-padded to 4608 rows)
WQS = 6                 # wq k-tiles per DMA slab (6 slabs of 6)
QC = NCOL // 4          # 160 fused columns per PSUM quadrant
DC = DN // 4            # 142 dense columns per PSUM quadrant
EG = (14, 14, 14, 14, 8)  # exp batch sizes over the 64 s-tiles

LAST_RESULT = None
_prog = None


def _build():
    nc = bacc.Bacc("TRN2", target_bir_lowering=False, debug=False,
                   num_devices=NCORES)

    # host-packed inputs (see kernel() below)
    hT = nc.dram_tensor("hT", [128, KT, U], BF16, kind="ExternalInput")
    wq = nc.dram_tensor("wq", [128, KT, NCOL], BF16, kind="ExternalInput")
    wd = nc.dram_tensor("wd", [128, KT, DN], BF16, kind="ExternalInput")
    kTc = nc.dram_tensor("kTc", [UPC, HD + 1, S], BF16, kind="ExternalInput")
    vc = nc.dram_tensor("vc", [UPC, 128, NT, HD + 1], BF16,
                        kind="ExternalInput")
    muT = nc.dram_tensor("muT", [HD, UPC, HD], F32, kind="ExternalInput")
    outc = nc.dram_tensor("outc", [U, DN], F32, kind="ExternalOutput")

    with tile.TileContext(nc) as tc:
        with (
            tc.tile_pool(name="const", bufs=1) as const,
            tc.tile_pool(name="wpool", bufs=3) as wpool,
            tc.tile_pool(name="kpool", bufs=3) as kpool,
            tc.tile_pool(name="vpool", bufs=3) as vpool,
            tc.tile_pool(name="ppool", bufs=2) as ppool,
            tc.tile_pool(name="upool", bufs=2) as upool,
            tc.tile_pool(name="pqpool", bufs=1, space="PSUM") as pqpool,
            tc.tile_pool(name="psc", bufs=3, space="PSUM") as pscpool,
            tc.tile_pool(name="pvpool", bufs=1, space="PSUM") as pvpool,
            tc.tile_pool(name="dram", bufs=1, space="DRAM") as dram,
        ):
            identity = const.tile([128, 128], F32)
            make_identity(nc, identity)

            # warm the PE p-state during the initial weight-DMA wait: ~3us of
            # continuous dummy matmuls take the clock 0.65 -> 2.4 GHz before
            # the first real QKV matmul lands
            wtile = const.tile([128, 128], BF16)
            nc.vector.memset(wtile, 0.0)
            ps_w = pscpool.tile([128, 2, 512], F32, tag="sg", name="ps_w")
            for w in range(30):
                nc.tensor.matmul(ps_w[0:1, 0, 0:128], wtile[:, 0:1],
                                 wtile[:, 0:128], start=True, stop=True)

            # ---------------- phase A: fused QKV projection ----------------
            hT_all = const.tile([128, KT, U], BF16)
            nc.sync.dma_start(out=hT_all, in_=hT[:, :, :])
            muT_sb = const.tile([HD, UPC, HD], F32)
            # gate behind the hT load: muT is only needed after the AllToAll
            nc.vector.tensor_copy(out=muT_sb[0:1, 0:1, 0:1],
                                  in_=hT_all[0:1, 0:1, 0:1])
            nc.scalar.dma_start(out=muT_sb, in_=muT[:, :, :])

            psQ = pqpool.tile([128, QC], F32, tag="bank", name="psQ")
            # slab sizes: first split 2+4 for an early start, last two small
            # (5+1) so the PE tail after the final transfer is 4 matmuls
            # instead of 24
            SLABS = [(0, 6), (6, 6), (12, 6), (18, 6), (24, 6), (30, 5),
                     (35, 1)]
            wslabs = []
            for si, (t0s, tns) in enumerate(SLABS):
                wslab = wpool.tile([128, tns, NCOL], BF16, tag="w",
                                   name="wslab", uniquify=True)
                if si == 0:
                    nc.sync.dma_start(out=wslab[:, 0:2, :],
                                      in_=wq[:, 0:2, :])
                    nc.sync.dma_start(out=wslab[:, 2:tns, :],
                                      in_=wq[:, 2:tns, :])
                else:
                    nc.sync.dma_start(out=wslab,
                                      in_=wq[:, t0s:t0s + tns, :])
                wslabs.append(wslab)
            for si, (t0s, tns) in enumerate(SLABS):
                for t6 in range(tns):
                    t = t0s + t6
                    lhs = hT_all[:, t, :]
                    for j in range(4):
                        nc.tensor.matmul(
                            psQ[32 * j:32 * j + 32, :], lhs,
                            wslabs[si][:, t6, QC * j:QC * (j + 1)],
                            start=(t == 0), stop=(t == KT - 1),
                            tile_position=(0, 32 * j))

            fq_sb = const.tile([128, QC], BF16)
            nc.scalar.activation(out=fq_sb, in_=psQ[:, :],
                                 func=mybir.ActivationFunctionType.Copy)

            fused_x = dram.tile([U, NCOL], BF16)
            fused_x_ji = bass.AP(
                tensor=fused_x.tensor, offset=fused_x.offset,
                ap=[[QC, 4], [NCOL, U], [1, QC]])
            # ACT-ring store: the SP ring is busy prefetching KV
            nc.scalar.dma_start(out=fused_x_ji, in_=fq_sb)
            # block d of the flat input (users 4d..4d+3) goes to core d
            fused_loc = dram.tile([NCORES, UPC, NCOL], BF16)
            nc.gpsimd.collective_compute(
                "AllToAll", mybir.AluOpType.bypass,
                replica_groups=[list(range(NCORES))],
                ins=[fused_x.opt()], outs=[fused_loc.opt()])

            # one strided gather for all 4 local users: (head, user, d)
            q_bf = const.tile([80, UPC, HD], BF16)
            for i in range(UPC):
                nc.scalar.dma_start(
                    out=q_bf[:, i, :],
                    in_=bass.AP(
                        tensor=fused_loc.tensor,
                        offset=fused_loc.offset + i * NCOL,
                        ap=[[UPC * NCOL, NCORES], [HD, HPC], [1, HD]]))
            q_f32 = const.tile([NH + 1, UPC, HD], F32)
            for i in range(UPC):
                nc.vector.tensor_copy(out=q_f32[:, i, :],
                                      in_=q_bf[0:NH + 1, i, :])
            wd_sb = const.tile([128, KT, DN], BF16)
            # dense-weight slabs 0/1: gated on the last q gather, racing the
            # user-3 KV loads for the post-A2A DMA slots
            for g in range(2):
                nc.vector.tensor_copy(
                    out=wd_sb[0:1, 9 * g:9 * g + 1, 0:1],
                    in_=q_bf[0:1, UPC - 1, 0:1])
                nc.sync.dma_start(
                    out=wd_sb[:, 9 * g:9 * (g + 1), :],
                    in_=wd[:, 9 * g:9 * (g + 1), :])
            vcur_all = const.tile([1, UPC, HD + 1], BF16)  # [v_cur | 1]
            nc.scalar.dma_start(
                out=vcur_all[:, :, 0:HD],
                in_=fused_loc[7, :, 2 * HD:3 * HD][None, :, :])
            nc.vector.memset(vcur_all[:, :, HD:HD + 1], 1.0)

            # ---------------- phase C: per-user flash-decode attention ------
            HIDP = KT * 128  # attn padded to 4608 so xbar tiles divide
            attn_c = dram.tile([UPC, HIDP], BF16, name="attn_c")
            zero4 = const.tile([UPC, HD], BF16)
            nc.vector.memset(zero4, 0.0)
            nc.scalar.dma_start(
                out=bass.AP(tensor=attn_c.tensor,
                            offset=attn_c.offset + HID,
                            ap=[[HIDP, UPC], [1, HD]]),
                in_=zero4)
            attn_ag = dram.tile([NCORES, UPC, HIDP], BF16,
                                addr_space="Shared", name="attn_ag")

            for i in range(UPC):
                # [k^T | mask row]: contraction row 64 carries the mask; the
                # q side puts 8.0 there so exp's 1/8 scale yields qk/8 + m
                kT_sb = kpool.tile([HD + 1, S], BF16, tag="kT", name="kT_sb")
                vones = vpool.tile([128, NT, HD + 1], BF16, tag="v",
                                   name="vones")
                if i < 3:
                    # guard: users 0/1/2 KV loads start once phase A's weight
                    # traffic is done (fq_sb written): they fill the AllToAll
                    # window on the otherwise idle DMA device
                    nc.vector.tensor_copy(out=kT_sb[0:1, 0:1],
                                          in_=fq_sb[0:1, 0:1])
                    nc.vector.tensor_copy(out=vones[0:1, 0:1, 0:1],
                                          in_=fq_sb[0:1, 0:1])
                else:
                    # user-3 KV right after the post-A2A q gathers
                    nc.vector.tensor_copy(out=kT_sb[0:1, 0:1],
                                          in_=q_bf[0:1, UPC - 1, 0:1])
                    nc.vector.tensor_copy(out=vones[0:1, 0:1, 0:1],
                                          in_=q_bf[0:1, UPC - 1, 0:1])

                nc.sync.dma_start(out=kT_sb, in_=kTc[i])
                nc.sync.dma_start(out=vones, in_=vc[i])
                # q heads 0..70 plus the shared k head at row 71, transposed
                ps_qT = pqpool.tile([HD, NH + 1], F32, tag="bank",
                                     name="ps_qT", uniquify=True)
                nc.tensor.transpose(ps_qT, q_f32[0:NH + 1, i, :],
                                    identity[0:NH + 1, 0:NH + 1])
                qkT = upool.tile([HD, NH + 1], F32, tag="qkT", name="qkT")
                nc.vector.tensor_copy(out=qkT, in_=ps_qT)

                # rotary as a matmul; row 64 = 8.0 scales the kT mask row
                ps_rot = pqpool.tile([HD, NH + 1], F32, tag="bank",
                                      name="ps_rot", uniquify=True)
                nc.tensor.matmul(ps_rot, muT_sb[:, i, :], qkT,
                                 start=True, stop=True)
                qTr = upool.tile([HD + 1, NH + 1], BF16, tag="qTr",
                                 name="qTr")
                nc.vector.tensor_copy(out=qTr[0:HD, :], in_=ps_rot)
                nc.vector.memset(qTr[HD:HD + 1, :], 8.0)

                # current-token score for all heads: [1, 71] (no mask row)
                ps_sc = pqpool.tile([1, NH], F32, tag="bank", name="ps_sc",
                                    uniquify=True)
                nc.tensor.matmul(ps_sc, qTr[0:HD, NH:NH + 1],
                                 qTr[0:HD, 0:NH], start=True, stop=True)
                curw = upool.tile([1, NH], BF16, tag="curw", name="curw")
                nc.scalar.activation(out=curw, in_=ps_sc,
                                     func=mybir.ActivationFunctionType.Exp,
                                     scale=0.125)

                # scores + exp over the 64 s-tiles, 7 tiles per PSUM bank at
                # stride 71, one batched exp per 2-bank group
                pT_all = ppool.tile([128, NT, NH], BF16, tag="pT",
                                    name="pT_all")
                pv = pvpool.tile([NH, HD + 1], F32, tag="pv", name="pv")
                t0 = 0
                for gi, gn in enumerate(EG):
                    ps_g = pscpool.tile([128, 2, 512], F32, tag="sg",
                                        name="ps_g")
                    for k in range(gn):
                        t = t0 + k
                        nc.tensor.matmul(
                            ps_g[:, k // 7, (k % 7) * NH:(k % 7 + 1) * NH],
                            kT_sb[:, t * 128:(t + 1) * 128],
                            qTr[:, 0:NH], start=True, stop=True)
                    if gn == 14:
                        nc.scalar.activation(
                            out=pT_all[:, t0:t0 + 14, :],
                            in_=ps_g[:, :, 0:7 * NH].rearrange(
                                "p b (k h) -> p b k h", h=NH),
                            func=mybir.ActivationFunctionType.Exp,
                            scale=0.125)
                    else:
                        for b in range((gn + 6) // 7):
                            bn = min(7, gn - 7 * b)
                            nc.scalar.activation(
                                out=pT_all[:, t0 + 7 * b:t0 + 7 * b + bn, :],
                                in_=ps_g[:, b, 0:bn * NH].rearrange(
                                    "p (k h) -> p k h", h=NH),
                                func=mybir.ActivationFunctionType.Exp,
                                scale=0.125)
                    t0 += gn

                # PV with fused row-sum via the ones column
                for t in range(NT):
                    nc.tensor.matmul(pv, pT_all[:, t, :], vones[:, t, :],
                                     start=(t == 0), stop=False)
                nc.tensor.matmul(pv, curw, vcur_all[:, i, :], start=False,
                                 stop=True)

                linv = upool.tile([NH, 1], F32, tag="linv", name="linv")
                nc.vector.reciprocal(out=linv, in_=pv[:, HD:HD + 1])
                attn_sb = upool.tile([NH, HD], BF16, tag="attn",
                                     name="attn_sb")
                nc.vector.tensor_scalar_mul(attn_sb, pv[:, 0:HD], linv)
                nc.scalar.dma_start(
                    out=bass.AP(tensor=attn_c.tensor,
                                offset=attn_c.offset + i * HIDP,
                                ap=[[HD, NH], [1, HD]]),
                    in_=attn_sb)

            # last dense-weight slab: bounce off the user-3 attn store so
            # its 3.6us transfer cannot delay the store (and the AllGather)
            bounce = const.tile([1, HD], BF16)
            nc.sync.dma_start(
                out=bounce,
                in_=bass.AP(tensor=attn_c.tensor,
                            offset=attn_c.offset + (UPC - 1) * HIDP,
                            ap=[[HD, 1], [1, HD]]))
            for g in (2, 3):
                nc.vector.tensor_copy(out=wd_sb[0:1, 9 * g:9 * g + 1, 0:1],
                                      in_=bounce[0:1, 0:1])
                nc.sync.dma_start(out=wd_sb[:, 9 * g:9 * (g + 1), :],
                                  in_=wd[:, 9 * g:9 * (g + 1), :])

            nc.gpsimd.collective_compute(
                "AllGather", mybir.AluOpType.bypass,
                replica_groups=[list(range(NCORES))],
                ins=[attn_c.opt()], outs=[attn_ag.opt()])

            # keep the PE p-state warm across the AllGather idle window so
            # the dense matmuls start at 2.4 GHz instead of 0.65 GHz
            ps_w2 = pscpool.tile([128, 2, 512], F32, tag="sg",
                                 name="ps_w2")
            for w in range(560):
                nc.tensor.matmul(ps_w2[0:1, 0, 0:128], wtile[:, 0:1],
                                 wtile[:, 0:128], start=True, stop=True)

            # ---------------- phase D: dense output projection --------------
            # attnT via two xbar DMA transposes of the gathered activations
            # (halves, so the dense matmuls start on k-tiles 0-17 while the
            # second half is still in flight)
            attnT = const.tile([128, KT, U], BF16)
            attn_flat = attn_ag.rearrange("c j n -> (c j) n")
            HK = KT // 2
            nc.sync.dma_start_transpose(
                out=attnT[:, 0:HK, :], in_=attn_flat[:, 0:HK * 128])
            nc.sync.dma_start_transpose(
                out=attnT[:, HK:KT, :], in_=attn_flat[:, HK * 128:KT * 128])

            psD = pqpool.tile([128, DC], F32, tag="bank", name="psD")
            for t in range(KT):
                for j in range(4):
                    nc.tensor.matmul(psD[32 * j:32 * j + 32, :],
                                     attnT[:, t, :],
                                     wd_sb[:, t, DC * j:DC * (j + 1)],
                                     start=(t == 0), stop=(t == KT - 1),
                                     tile_position=(0, 32 * j))

            outD = const.tile([128, DC], F32)
            nc.vector.tensor_copy(out=outD, in_=psD[:, :])
            outc_ji = bass.AP(
                tensor=outc.ap().tensor, offset=0,
                ap=[[DC, 4], [DN, U], [1, DC]])
            nc.scalar.dma_start(out=outc_ji, in_=outD)

    nc.compile()
    return nc


def _rot_mat(cos_u, sin_u):
    """M such that M @ x = x*cos + rotate_half(x)*sin, for one user."""
    m = np.zeros((HD, HD), np.float32)
    np.fill_diagonal(m, cos_u)
    half = HD // 2
    for r in range(half):
        m[r, r + half] += -sin_u[r]
        m[r + half, r] += sin_u[r + half]
    return m


def kernel(hidden_states, cos, sin, k_cache, v_cache, attn_masks, w_qkv,
           w_dense, trace=False):
    global _prog, LAST_RESULT
    import ml_dtypes

    bf16 = ml_dtypes.bfloat16
    if _prog is None:
        _prog = _build()

    hidden_states = np.asarray(hidden_states, np.float32)
    cos = np.asarray(cos, np.float32)
    sin = np.asarray(sin, np.float32)
    k_cache = np.asarray(k_cache, np.float32)
    v_cache = np.asarray(v_cache, np.float32)
    attn_masks = np.asarray(attn_masks, np.float32)
    w_qkv = np.asarray(w_qkv, np.float32)
    w_dense = np.asarray(w_dense, np.float32)

    def pack_k(m, ncol):
        """[4544, ncol] -> [128, 36, ncol] bf16, zero-padded to 4608 rows."""
        p = np.zeros((KT * 128, ncol), np.float32)
        p[:m.shape[0]] = m
        return np.ascontiguousarray(
            p.reshape(KT, 128, ncol).transpose(1, 0, 2).astype(bf16))

    hT = pack_k(hidden_states[0].T, U)                       # [128, 36, 32]
    wqT = np.zeros((HID, NCORES * NCOL), np.float32)
    wqT[:, :w_qkv.shape[0]] = w_qkv.T
    wdT = w_dense.T                                          # [4544, 4544]

    in_maps = []
    for c in range(NCORES):
        us = slice(UPC * c, UPC * (c + 1))
        k_u = np.moveaxis(k_cache[:, 0, us], 1, 0).reshape(UPC, S, HD)
        m_u = np.moveaxis(attn_masks[:, 0, us], 1, 0).reshape(UPC, S)
        kT_u = np.concatenate(
            [np.transpose(k_u, (0, 2, 1)), m_u[:, None, :]], axis=1)
        v_u = np.moveaxis(v_cache[:, 0, us], 1, 0).reshape(UPC, NT, 128, HD)
        vones = np.concatenate(
            [v_u, np.ones((UPC, NT, 128, 1), np.float32)], axis=3)
        muT = np.stack([
            _rot_mat(cos[0, u, 0], sin[0, u, 0]).T
            for u in range(UPC * c, UPC * (c + 1))
        ])                                                   # [4, 64, 64]
        in_maps.append({
            "hT": hT,
            "wq": pack_k(wqT[:, NCOL * c:NCOL * (c + 1)], NCOL),
            "wd": pack_k(wdT[:, DN * c:DN * (c + 1)], DN),
            "kTc": np.ascontiguousarray(kT_u.astype(bf16)),
            "vc": np.ascontiguousarray(
                vones.transpose(0, 2, 1, 3).astype(bf16)),
            "muT": np.ascontiguousarray(
                np.transpose(muT, (1, 0, 2)).astype(np.float32)),
        })

    res = run_bass_kernel_spmd(_prog, in_maps, list(range(NCORES)),
                               trace=trace)
    LAST_RESULT = res
    out = np.concatenate([res.results[c]["outc"] for c in range(NCORES)],
                         axis=1)                             # [32, 4544]
    return out[None].astype(np.float32)
